# revision 33
# baseline (speedup 1.0000x reference)
"""Differentiable AAC forward pass on 8 Trainium2 NeuronCores.

Strategy: data-parallel over the batch dim (8 batches -> 8 cores).

v3 design:
- Host pre-transposes + pre-windows the framed audio into four
  chunk-major arrays (fwd/rev x window-half), so the MDCT fold is 2
  tensor ops per channel, no on-device transposes or window mults.
- The B-half sign is folded into negated rows of the DCT-IV matrix.
- MDCT matmul in f32r (1 cycle/row at N=512) instead of fp32 (4x).
- Both channels fused into (128, 2, 1024) tiles for all elementwise
  work: half the instruction count, no per-iteration channel join.
- Gain-search bit count: tensor_scalar with accum_out does the
  exponent-shift + full-frame reduction in one op; iterations
  alternate DVE <-> ACT+Pool so consecutive iterations pipeline.
- Ln/Exp/Identity all served by the natural_log_exp_and_others act
  table (patched table list below stops the reload thrash).
- IMDCT matrices + ring are bf16 (error lands post-quantization).
"""

import functools

import numpy as np
import ml_dtypes

import concourse.bass as bass
import concourse.bacc as bacc
import concourse.mybir as mybir
import concourse.tile as tile
from concourse.bass_utils import run_bass_kernel_spmd


def _patch_act_tables():
    """Make ln/exp resolvable only through natural_log_exp_and_others, so
    the act-table-load insertion pass picks the combined table instead of
    thrashing between the ln-only and exp-only sets (1.3us per reload).
    Indices (act_func_set_ids) are preserved; the chosen id stays valid
    for walrus / the real act_info.json."""
    import concourse.hw_specs as hw_specs

    orig = hw_specs.get_activation_tables
    if getattr(orig, "_aac_patched", False):
        return

    @functools.cache
    def patched(module_arch):
        tabs = orig(module_arch)
        ln_exp = {mybir.ActivationFunctionType.Ln,
                  mybir.ActivationFunctionType.Exp}
        out = {}
        for name, funcs in tabs.items():
            if name == "natural_log_exp_and_others":
                out[name] = set(funcs)
            else:
                out[name] = set(funcs) - ln_exp
        return out

    patched._aac_patched = True
    hw_specs.get_activation_tables = patched
    bacc.get_activation_tables = patched
    import concourse.bass_interp as bass_interp
    bass_interp.get_activation_tables = patched


_patch_act_tables()

M = 1024
M2 = 2 * M                  # both channels fused on the free dim
N2 = 2048
NCORES = 8
MAGIC = 12582912.0          # 1.5 * 2^23, RNE-to-integer magic for |v| < 2^22
LN2 = 0.6931471805599453
TARGET_BITS = 128000 * 1024 / 48000.0   # 2730.666... bits per frame
SIGN_MASK = -2147483648     # 0x80000000 as int32
ABS_MASK = 0x7FFFFFFF

NEWTON = True               # refine ax75 with one Newton step


def _round_mant(x, bits=11):
    """Round fp32 array to `bits` explicit mantissa bits (RNE) == f32r."""
    x = np.ascontiguousarray(x, np.float32)
    xi = x.view(np.uint32).astype(np.uint64)
    shift = 23 - bits
    add = (np.uint64(1) << np.uint64(shift - 1)) - np.uint64(1)
    lsb = (xi >> np.uint64(shift)) & np.uint64(1)
    xi = (xi + add + lsb) >> np.uint64(shift) << np.uint64(shift)
    return xi.astype(np.uint32).view(np.float32)


def host_constants():
    """DCT-IV basis (B-half rows negated, f32r), bf16 IMDCT matrices,
    and the fp32 identity for transposes."""
    n = np.arange(N2, dtype=np.float64)
    w = np.sin(np.pi / N2 * (n + 0.5))
    k = np.arange(M, dtype=np.float64)
    j = np.arange(M, dtype=np.float64)
    C4 = np.cos(np.pi / M * np.outer(j + 0.5, k + 0.5))          # (M j, M k)
    C4s = C4.copy()
    C4s[:512] *= -1.0         # sign of the B-half fold
    Cm = np.cos(np.pi / M * np.outer(n + 0.5 + M / 2, k + 0.5))  # (N2, M)
    Cw2 = (2.0 / M) * (w[:, None] * Cm)                          # (N2, M)
    R1 = Cw2[:M].T        # (M k, M r): A-half  td[:, r]
    R2 = Cw2[M:].T        # (M k, M r): B-half  td[:, 1024+r]

    def lay(a):  # (1024, 1024) -> (128, 8, 1024) [p, t, c] = a[t*128+p, c]
        return np.ascontiguousarray(
            a.astype(np.float32).reshape(8, 128, M).transpose(1, 0, 2))

    g = np.arange(121, dtype=np.float64)
    consts = {
        "c4": _round_mant(lay(C4s)),
        "r1": lay(R1).astype(ml_dtypes.bfloat16),
        "r2": lay(R2).astype(ml_dtypes.bfloat16),
        "ident": np.eye(128, dtype=ml_dtypes.bfloat16),
        "iota121": np.broadcast_to(g.astype(np.float32), (128, 121)).copy(),
        "lutinv": np.broadcast_to(
            np.exp2(-0.75 * g / 4.0).astype(np.float32), (128, 121)).copy(),
    }
    return consts


def host_inputs(audio_c):
    """Per-core input prep: four windowed chunk-major arrays (2,4,128,F).

    W1F[c,q,p,f] = x[c,f,  q*128+p       ] * wa[q*128+p]
    W1R[c,q,p,f] = x[c,f,  1023-(q*128+p)] * wa[1023-(q*128+p)]
    W2F[c,q,p,f] = x[c,f+1, 512+q*128+p  ] * wb[512+q*128+p]
    W2R[c,q,p,f] = x[c,f+1, 511-(q*128+p)] * wb[511-(q*128+p)]

    s chunks (for the folded DCT-IV):
      s[:, 0:4, :]  (B-half, sign in C4s) = W2R + W2F
      s[:, 4:8, :]  (A-half)              = W1F - W1R
    """
    C, T = audio_c.shape
    F = -(-(T + M) // M)
    nrows = F + 1
    x = np.zeros((C, nrows * M), np.float32)
    x[:, M:M + T] = audio_c
    x = x.reshape(C, nrows, M)

    n = np.arange(N2, dtype=np.float64)
    w = np.sin(np.pi / N2 * (n + 0.5)).astype(np.float32)
    wa, wb = w[:M], w[M:]

    xc = x[:, :F, :]          # (C, F, 1024) frame f <- row f
    xn = x[:, 1:F + 1, :]     # frame f <- row f+1

    def chunkmajor(a):        # (C, F, 512) -> (C, 4, 128, F)
        return np.ascontiguousarray(
            a.reshape(C, F, 4, 128).transpose(0, 2, 3, 1))

    w1f = chunkmajor(xc[:, :, :512] * wa[:512])
    w1r = chunkmajor(xc[:, :, ::-1][:, :, :512] * wa[::-1][:512])
    w2f = chunkmajor(xn[:, :, 512:] * wb[512:])
    w2r = chunkmajor((xn[:, :, ::-1] * wb[::-1])[:, :, 512:])
    return {"w1f": w1f, "w1r": w1r, "w2f": w2f, "w2r": w2r}, F


def build_nc(nb, ncores=NCORES):
    """Build the per-core Bass kernel. nb: number of 128-frame blocks."""
    F = nb * 128
    out_len = F * M

    nc = bacc.Bacc("TRN2", target_bir_lowering=False, debug=False,
                   num_devices=ncores)
    f32 = mybir.dt.float32
    f32r = mybir.dt.float32r
    bf16 = mybir.dt.bfloat16
    i32 = mybir.dt.int32
    i16 = mybir.dt.int16
    i8 = mybir.dt.int8
    Alu = mybir.AluOpType
    Act = mybir.ActivationFunctionType

    w1f_d = nc.dram_tensor("w1f", [2, 4, 128, F], f32, kind="ExternalInput")
    w1r_d = nc.dram_tensor("w1r", [2, 4, 128, F], f32, kind="ExternalInput")
    w2f_d = nc.dram_tensor("w2f", [2, 4, 128, F], f32, kind="ExternalInput")
    w2r_d = nc.dram_tensor("w2r", [2, 4, 128, F], f32, kind="ExternalInput")
    c4_d = nc.dram_tensor("c4", [128, 8, M], f32r, kind="ExternalInput")
    r1_d = nc.dram_tensor("r1", [128, 8, M], bf16, kind="ExternalInput")
    r2_d = nc.dram_tensor("r2", [128, 8, M], bf16, kind="ExternalInput")
    id_d = nc.dram_tensor("ident", [128, 128], bf16, kind="ExternalInput")
    io_d = nc.dram_tensor("iota121", [128, 121], f32, kind="ExternalInput")
    lv_d = nc.dram_tensor("lutinv", [128, 121], f32, kind="ExternalInput")
    out_d = nc.dram_tensor("out", [2, out_len], f32, kind="ExternalOutput")

    def w_slice(t, c, b):
        # (128, 4, 128) tile: [p, q, f] = t[c, q, p, b*128+f]
        return bass.AP(tensor=t, offset=(c * 4 * 128 + 0) * F + b * 128,
                       ap=[[F, 128], [128 * F, 4], [1, 128]])

    def out_slice(c, blk0, npart, r0, nr):
        return bass.AP(tensor=out_d, offset=c * out_len + blk0 * M + r0,
                       ap=[[M, npart], [1, nr]])

    # integer threshold: bits > TARGET  <=>  sum(E) > TARGET + 125*2048
    thresh = float(int(np.floor(TARGET_BITS + 125 * 2048))) + 0.5  # 258730.5

    with tile.TileContext(nc) as tc:
        import contextlib
        ctx = contextlib.ExitStack()
        with ctx:
            consts = ctx.enter_context(tc.tile_pool(name="consts", bufs=1))
            xin = ctx.enter_context(tc.tile_pool(name="xin", bufs=2))
            srp = ctx.enter_context(tc.tile_pool(name="srp", bufs=2))
            abp = ctx.enter_context(tc.tile_pool(name="abp", bufs=1))
            axp = ctx.enter_context(tc.tile_pool(name="axp", bufs=3))
            sbp = ctx.enter_context(tc.tile_pool(name="sbp", bufs=2))
            zp = ctx.enter_context(tc.tile_pool(name="zp", bufs=2))
            escr = ctx.enter_context(tc.tile_pool(name="escr", bufs=2))
            scr = ctx.enter_context(tc.tile_pool(name="scr", bufs=3))
            ax0p = ctx.enter_context(tc.tile_pool(name="ax0p", bufs=2))
            dfp = ctx.enter_context(tc.tile_pool(name="dfp", bufs=2))
            dqtp = ctx.enter_context(tc.tile_pool(name="dqtp", bufs=2))
            outp = ctx.enter_context(tc.tile_pool(name="outp", bufs=2))
            stat = ctx.enter_context(tc.tile_pool(name="stat", bufs=5))
            lutp = ctx.enter_context(tc.tile_pool(name="lutp", bufs=2))
            psM = ctx.enter_context(tc.tile_pool(name="psM", bufs=3, space="PSUM"))
            psQ = ctx.enter_context(tc.tile_pool(name="psQ", bufs=2, space="PSUM"))
            psI = ctx.enter_context(tc.tile_pool(name="psI", bufs=2, space="PSUM"))

            c4_sb = consts.tile([128, 8, M], f32r)
            nc.sync.dma_start(out=c4_sb[:, 0:4, :], in_=c4_d[:, 0:4, :])
            nc.sync.dma_start(out=c4_sb[:, 4:8, :], in_=c4_d[:, 4:8, :])
            r1_sb = consts.tile([128, 8, M], bf16)
            r2_sb = consts.tile([128, 8, M], bf16)
            id_sb = consts.tile([128, 128], bf16)
            nc.sync.dma_start(out=id_sb, in_=id_d[:, :])
            io_sb = consts.tile([128, 121], f32)
            nc.sync.dma_start(out=io_sb, in_=io_d[:, :])
            lv_sb = consts.tile([128, 121], f32)
            nc.sync.dma_start(out=lv_sb, in_=lv_d[:, :])
            eps35 = consts.tile([128, 1], f32)
            nc.vector.memset(eps35, 1e-35)
            zbf = consts.tile([128, 1], bf16)
            nc.vector.memset(zbf, 0.0)
            half05 = consts.tile([128, 1], f32)
            nc.vector.memset(half05, 0.5)
            zf32 = consts.tile([128, 1], f32)
            nc.vector.memset(zf32, 0.0)
            magicb = consts.tile([128, 1], f32)
            nc.vector.memset(magicb, MAGIC)

            # dqT ring: [parity][channel] -> tile (128, 8, 129) bf16
            dqt_ring = [[None, None], [None, None]]

            def mdct_block(b):
                """Returns (sb2, ax2) fused tiles (128, 2, M) for block b."""
                sb2 = sbp.tile([128, 2, M], bf16, name=f"sb_{b}", tag="sb")
                ab2 = abp.tile([128, 2, M], f32, name=f"ab_{b}", tag="ab")
                for c in range(2):
                    tw1f = xin.tile([128, 4, 128], f32, name=f"w1f_{b}_{c}",
                                    tag="xin")
                    nc.sync.dma_start(out=tw1f, in_=w_slice(w1f_d, c, b))
                    tw1r = xin.tile([128, 4, 128], f32, name=f"w1r_{b}_{c}",
                                    tag="xin")
                    nc.sync.dma_start(out=tw1r, in_=w_slice(w1r_d, c, b))
                    tw2f = xin.tile([128, 4, 128], f32, name=f"w2f_{b}_{c}",
                                    tag="xin")
                    nc.sync.dma_start(out=tw2f, in_=w_slice(w2f_d, c, b))
                    tw2r = xin.tile([128, 4, 128], f32, name=f"w2r_{b}_{c}",
                                    tag="xin")
                    nc.sync.dma_start(out=tw2r, in_=w_slice(w2r_d, c, b))

                    sr = srp.tile([128, 8, 128], f32r, name=f"sr_{b}_{c}",
                                  tag="sr")
                    nc.vector.tensor_add(out=sr[:, 0:4, :], in0=tw2r, in1=tw2f)
                    nc.vector.tensor_sub(out=sr[:, 4:8, :], in0=tw1f, in1=tw1r)

                    for kc in range(2):
                        psm = psM.tile([128, 512], f32, name=f"psm_{b}_{c}_{kc}",
                                       tag="psm")
                        for jt in range(8):
                            nc.tensor.matmul(psm, sr[:, jt, :],
                                             c4_sb[:, jt, kc * 512:(kc + 1) * 512],
                                             start=(jt == 0), stop=(jt == 7))
                        ks = slice(kc * 512, (kc + 1) * 512)
                        nc.scalar.activation(out=ab2[:, c, ks], in_=psm,
                                             func=Act.Abs)
                        nc.scalar.activation(out=sb2[:, c, ks], in_=psm,
                                             func=Act.Sign, bias=zf32)

                ln = scr.tile([128, 2, M], f32, name=f"ln_{b}", tag="scr")
                nc.scalar.activation(out=ln, in_=ab2, func=Act.Ln, bias=eps35)
                ax2 = axp.tile([128, 2, M], f32, name=f"ax_{b}", tag="ax")
                if NEWTON:
                    ax0 = ax0p.tile([128, 2, M], f32, name=f"ax0_{b}",
                                    tag="ax0")
                    nc.scalar.activation(out=ax0, in_=ln, func=Act.Exp,
                                         scale=0.75)
                    # one Newton step on a^4 = |c|^3:
                    #   a' = 0.75 a + 0.25 (|c|/a)^3
                    rcp = scr.tile([128, 2, M], f32, name=f"rcp_{b}",
                                   tag="scr")
                    nc.vector.reciprocal(out=rcp, in_=ax0)
                    tt = scr.tile([128, 2, M], f32, name=f"tt_{b}", tag="scr")
                    nc.gpsimd.tensor_mul(out=tt, in0=ab2, in1=rcp)
                    t2 = scr.tile([128, 2, M], f32, name=f"t2_{b}", tag="scr")
                    nc.scalar.activation(out=t2, in_=tt, func=Act.Square)
                    v3 = scr.tile([128, 2, M], f32, name=f"v3_{b}", tag="scr")
                    nc.vector.scalar_tensor_tensor(out=v3, in0=t2, scalar=0.25,
                                                   in1=tt, op0=Alu.mult,
                                                   op1=Alu.mult)
                    nc.vector.scalar_tensor_tensor(out=ax2, in0=ax0,
                                                   scalar=0.75, in1=v3,
                                                   op0=Alu.mult, op1=Alu.add)
                else:
                    ax0 = ax2
                    nc.scalar.activation(out=ax2, in_=ln, func=Act.Exp,
                                         scale=0.75)
                return sb2, ax2, ax0

            def lut_pow2(prefix, b, g_ap, tag):
                """inv = 2^{-3 g/16} exactly via one-hot LUT: 2 DVE ops."""
                eq = lutp.tile([128, 121], f32, name=f"{prefix}eq_{b}",
                               tag=f"{tag}a")
                nc.vector.tensor_scalar(out=eq, in0=io_sb, scalar1=g_ap,
                                        scalar2=None, op0=Alu.is_equal)
                eqs = lutp.tile([128, 121], f32, name=f"{prefix}es_{b}",
                                tag=f"{tag}b")
                nc.vector.tensor_mul(out=eqs, in0=eq, in1=lv_sb)
                inv = stat.tile([128, 1], f32, name=f"{prefix}iv_{b}",
                                tag=f"{tag}c")
                with nc.allow_low_precision(reason="one-hot dot"):
                    nc.vector.tensor_scalar(
                        out=eqs, in0=eqs, scalar1=1.0, scalar2=None,
                        op0=Alu.mult, op1=Alu.add, accum_out=inv)
                return inv

            def search_block(b, ax2, ax0):
                """8-iter integer binary search; returns gains (hi) (128,1)."""
                lo = stat.tile([128, 1], f32, name=f"lo_{b}", tag="lo")
                nc.vector.memset(lo, 0.0)
                hi = stat.tile([128, 1], f32, name=f"hi_{b}", tag="hi")
                nc.vector.memset(hi, 120.0)
                for it in range(8):
                    axu = ax0 if it < 2 else ax2
                    t = stat.tile([128, 1], f32, name=f"t_{b}_{it}", tag="st1")
                    nc.vector.tensor_add(out=t, in0=lo, in1=hi)
                    mid = stat.tile([128, 1], f32, name=f"mid_{b}_{it}",
                                    tag="st2")
                    nc.vector.tensor_scalar(out=mid, in0=t, scalar1=0.5,
                                            scalar2=-0.25, op0=Alu.mult,
                                            op1=Alu.add)
                    nc.vector.tensor_scalar(out=mid, in0=mid, scalar1=MAGIC,
                                            scalar2=MAGIC, op0=Alu.add,
                                            op1=Alu.subtract)
                    inv = lut_pow2(f"s{it}_", b, mid, "lp")
                    z = zp.tile([128, 2, M], f32, name=f"z_{b}_{it}", tag="z")
                    ezt = escr.tile([128, 2, M], i16, name=f"e_{b}_{it}",
                                    tag="ez")
                    # z halves off-DVE (Pool / ACT); exponent extraction =
                    # high-i16 halves >> 7, then one fused arith accumulate,
                    # both on DVE (Pool has no shift; bitwise can't fuse
                    # with the arith reduce in one op).
                    nc.gpsimd.tensor_scalar(out=z[:, 0, :], in0=axu[:, 0, :],
                                            scalar1=inv, scalar2=0.5,
                                            op0=Alu.mult, op1=Alu.add)
                    nc.scalar.activation(out=z[:, 1, :], in_=axu[:, 1, :],
                                         func=Act.Identity, scale=inv,
                                         bias=half05)
                    zi16 = z.bitcast(i16)   # (128, 2, 2M)
                    es = stat.tile([128, 1], f32, name=f"es_{b}_{it}",
                                   tag="es")
                    with nc.allow_low_precision(reason="exponent bits"):
                        nc.vector.tensor_scalar(
                            out=ezt, in0=zi16[:, :, 1:2 * M:2],
                            scalar1=7, scalar2=None,
                            op0=Alu.logical_shift_right)
                        nc.vector.tensor_scalar(
                            out=ezt, in0=ezt, scalar1=1,
                            scalar2=None, op0=Alu.mult, op1=Alu.add,
                            accum_out=es)
                    msk = stat.tile([128, 1], i32, name=f"mk_{b}_{it}",
                                    tag="st6")
                    with nc.allow_low_precision(reason="int mask"):
                        nc.vector.tensor_scalar(out=msk, in0=es,
                                                scalar1=thresh,
                                                scalar2=None, op0=Alu.is_gt)
                        mskn = stat.tile([128, 1], i32, name=f"mn_{b}_{it}",
                                         tag="st7")
                        nc.vector.tensor_scalar(out=mskn, in0=msk, scalar1=-1,
                                                scalar2=1, op0=Alu.mult,
                                                op1=Alu.add)
                    mp1 = stat.tile([128, 1], f32, name=f"mp_{b}_{it}",
                                    tag="st8")
                    nc.vector.tensor_scalar(out=mp1, in0=mid, scalar1=1.0,
                                            scalar2=None, op0=Alu.add)
                    # lo = too_big ? mid+1 : lo ; hi = too_big ? hi : mid
                    nc.vector.copy_predicated(out=lo, mask=msk, data=mp1)
                    nc.vector.copy_predicated(out=hi, mask=mskn, data=mid)
                return hi

            def quant_block(b, gains, ax2, sb2):
                """Quantize+dequantize; returns fused df (128, 2, M) i32
                (= fp32 bit pattern of signed dq).

                q_soft = ax75 * 2^{-3g/16}; q1 = q_soft + MAGIC holds
                MAGIC + round(q_soft) exactly. a43 = qm^{4/3} * 2^{g/4} via
                one Exp with per-partition bias g*ln2/4."""
                inv2 = lut_pow2("q_", b, gains, "lq")
                gb = stat.tile([128, 1], f32, name=f"gb_{b}", tag="st5")
                nc.vector.tensor_scalar(out=gb, in0=gains, scalar1=LN2 / 4.0,
                                        scalar2=None, op0=Alu.mult)
                q1 = scr.tile([128, 2, M], f32, name=f"q1_{b}", tag="scr")
                nc.scalar.activation(out=q1, in_=ax2, func=Act.Identity,
                                     scale=inv2, bias=magicb)
                qm = scr.tile([128, 2, M], f32, name=f"qm_{b}", tag="scr")
                nc.vector.tensor_scalar(out=qm, in0=q1, scalar1=MAGIC,
                                        scalar2=0.5, op0=Alu.subtract,
                                        op1=Alu.max)
                lq = scr.tile([128, 2, M], f32, name=f"lq_{b}", tag="scr")
                nc.scalar.activation(out=lq, in_=qm, func=Act.Ln)
                a43 = scr.tile([128, 2, M], f32, name=f"a43_{b}", tag="scr")
                nc.scalar.activation(out=a43, in_=lq, func=Act.Exp,
                                     scale=4.0 / 3.0, bias=gb)
                mq = scr.tile([128, 2, M], f32, name=f"mq_{b}", tag="scr")
                nc.gpsimd.tensor_scalar(out=mq, in0=q1, scalar1=MAGIC + 0.5,
                                        scalar2=None, op0=Alu.is_gt)
                d2 = scr.tile([128, 2, M], f32, name=f"d2_{b}", tag="scr")
                nc.gpsimd.tensor_mul(out=d2, in0=a43, in1=mq)
                df = dfp.tile([128, 2, M], bf16, name=f"df_{b}", tag="df")
                nc.gpsimd.tensor_mul(out=df, in0=d2, in1=sb2)
                return df

            def dqt_block(b, df):
                """Transpose signed dq into the bf16 dqT ring; write sliver
                col 128 of block b-1's buffers."""
                par = b % 2
                dqf = df
                for c in range(2):
                    buf = dqtp.tile([128, 8, 129], bf16, name=f"dqt_{b}_{c}",
                                    tag=f"dqt{c}")
                    dqt_ring[par][c] = buf
                    for kt in range(8):
                        psq = psQ.tile([128, 128], bf16, name=f"psq_{b}_{c}_{kt}",
                                       tag="psq")
                        nc.tensor.transpose(
                            psq, dqf[:, c, kt * 128:(kt + 1) * 128], id_sb)
                        if kt % 2 == 0:
                            nc.vector.tensor_copy(out=buf[:, kt, 0:128],
                                                  in_=psq)
                        else:
                            nc.scalar.activation(out=buf[:, kt, 0:128],
                                                 in_=psq, func=Act.Copy)
                        if b > 0:
                            prev = dqt_ring[1 - par][c]
                            nc.scalar.activation(out=prev[:, kt, 128:129],
                                                 in_=psq[:, 0:1],
                                                 func=Act.Copy)

            def imdct_block(bp):
                """IMDCT + fused OLA for out blocks [bp*128, bp*128+128)."""
                par = bp % 2
                for c in range(2):
                    buf = dqt_ring[par][c]
                    for rc in range(2):
                        psr = psI.tile([128, 512], f32, name=f"psr_{bp}_{c}_{rc}",
                                       tag="psr")
                        for kt in range(8):
                            nc.tensor.matmul(
                                psr, buf[:, kt, 0:128],
                                r2_sb[:, kt, rc * 512:(rc + 1) * 512],
                                start=(kt == 0), stop=False)
                        for kt in range(8):
                            nc.tensor.matmul(
                                psr, buf[:, kt, 1:129],
                                r1_sb[:, kt, rc * 512:(rc + 1) * 512],
                                start=False, stop=(kt == 7))
                        ot = outp.tile([128, 512], f32, name=f"ot_{bp}_{c}_{rc}",
                                       tag="ot")
                        nc.scalar.activation(out=ot, in_=psr, func=Act.Copy)
                        nc.sync.dma_start(
                            out=out_slice(c, bp * 128, 128, rc * 512, 512),
                            in_=ot)

            # software pipeline: emit block b+1's front-end before block
            # b's back-end so the in-order PE queue isn't head-of-line
            # blocked by imdct(b-1) (which waits on block b's sliver).
            state = mdct_block(0)
            # IMDCT matrices aren't needed until imdct(0) at iteration 1;
            # keep their 12us of DMA behind the first block's inputs.
            nc.sync.dma_start(out=r1_sb, in_=r1_d[:, :, :])
            nc.sync.dma_start(out=r2_sb, in_=r2_d[:, :, :])
            for b in range(nb):
                nxt = mdct_block(b + 1) if b + 1 < nb else None
                sb2, ax2, ax0 = state
                gains = search_block(b, ax2, ax0)
                df = quant_block(b, gains, ax2, sb2)
                dqt_block(b, df)
                if b > 0:
                    imdct_block(b - 1)
                state = nxt
            # final sliver = 0 (frame F does not exist), then last IMDCT
            par = (nb - 1) % 2
            for c in range(2):
                for kt in range(8):
                    nc.gpsimd.tensor_copy(out=dqt_ring[par][c][:, kt, 128:129],
                                          in_=zbf)
            imdct_block(nb - 1)

    nc.compile()
    return nc


_CACHE = {}


def _get_nc(nb, ncores):
    key = (nb, ncores)
    if key not in _CACHE:
        _CACHE[key] = (build_nc(nb, ncores), host_constants())
    return _CACHE[key]


def run(audio, trace=False):
    """audio (B, C, T) float32 -> (out (B, C, T) float32, results obj)."""
    B, C, T = audio.shape
    assert C == 2
    F = -(-(T + M) // M)
    nb = F // 128
    assert nb * 128 == F, "frame count must be a multiple of 128"

    nc, consts = _get_nc(nb, B)

    audio = np.ascontiguousarray(audio, np.float32)
    in_maps = []
    for core in range(B):
        arrs, _ = host_inputs(audio[core])
        in_maps.append({**arrs, **consts})

    res = run_bass_kernel_spmd(nc, in_maps, core_ids=list(range(B)),
                               trace=trace)
    out = np.stack([r["out"][:, :T] for r in res.results])
    return out, res


def kernel(audio):
    return run(audio)[0]


# revision 34
# speedup vs baseline: 1.4056x; 1.4056x over previous
"""Differentiable AAC forward pass on 8 Trainium2 NeuronCores.

Strategy: data-parallel over the batch dim (8 batches -> 8 cores).

v3 design:
- Host pre-transposes + pre-windows the framed audio into four
  chunk-major arrays (fwd/rev x window-half), so the MDCT fold is 2
  tensor ops per channel, no on-device transposes or window mults.
- The B-half sign is folded into negated rows of the DCT-IV matrix.
- MDCT matmul in f32r (1 cycle/row at N=512) instead of fp32 (4x).
- Both channels fused into (128, 2, 1024) tiles for all elementwise
  work: half the instruction count, no per-iteration channel join.
- Gain-search bit count: tensor_scalar with accum_out does the
  exponent-shift + full-frame reduction in one op; iterations
  alternate DVE <-> ACT+Pool so consecutive iterations pipeline.
- Ln/Exp/Identity all served by the natural_log_exp_and_others act
  table (patched table list below stops the reload thrash).
- IMDCT matrices + ring are bf16 (error lands post-quantization).
"""

import functools

import numpy as np
import ml_dtypes

import concourse.bass as bass
import concourse.bacc as bacc
import concourse.mybir as mybir
import concourse.tile as tile
from concourse.bass_utils import run_bass_kernel_spmd


def _patch_act_tables():
    """Make ln/exp resolvable only through natural_log_exp_and_others, so
    the act-table-load insertion pass picks the combined table instead of
    thrashing between the ln-only and exp-only sets (1.3us per reload).
    Indices (act_func_set_ids) are preserved; the chosen id stays valid
    for walrus / the real act_info.json."""
    import concourse.hw_specs as hw_specs

    orig = hw_specs.get_activation_tables
    if getattr(orig, "_aac_patched", False):
        return

    @functools.cache
    def patched(module_arch):
        tabs = orig(module_arch)
        ln_exp = {mybir.ActivationFunctionType.Ln,
                  mybir.ActivationFunctionType.Exp}
        out = {}
        for name, funcs in tabs.items():
            if name == "natural_log_exp_and_others":
                out[name] = set(funcs)
            else:
                out[name] = set(funcs) - ln_exp
        return out

    patched._aac_patched = True
    hw_specs.get_activation_tables = patched
    bacc.get_activation_tables = patched
    import concourse.bass_interp as bass_interp
    bass_interp.get_activation_tables = patched


_patch_act_tables()

M = 1024
M2 = 2 * M                  # both channels fused on the free dim
N2 = 2048
NCORES = 8
MAGIC = 12582912.0          # 1.5 * 2^23, RNE-to-integer magic for |v| < 2^22
LN2 = 0.6931471805599453
TARGET_BITS = 128000 * 1024 / 48000.0   # 2730.666... bits per frame
SIGN_MASK = -2147483648     # 0x80000000 as int32
ABS_MASK = 0x7FFFFFFF

NEWTON = False              # HW Ln/Exp tables measure 1.2e-5 max rel err;
                            # the Newton step is not worth its serial chain


def _round_mant(x, bits=11):
    """Round fp32 array to `bits` explicit mantissa bits (RNE) == f32r."""
    x = np.ascontiguousarray(x, np.float32)
    xi = x.view(np.uint32).astype(np.uint64)
    shift = 23 - bits
    add = (np.uint64(1) << np.uint64(shift - 1)) - np.uint64(1)
    lsb = (xi >> np.uint64(shift)) & np.uint64(1)
    xi = (xi + add + lsb) >> np.uint64(shift) << np.uint64(shift)
    return xi.astype(np.uint32).view(np.float32)


def host_constants():
    """DCT-IV basis (B-half rows negated, f32r), bf16 IMDCT matrices,
    and the fp32 identity for transposes."""
    n = np.arange(N2, dtype=np.float64)
    w = np.sin(np.pi / N2 * (n + 0.5))
    k = np.arange(M, dtype=np.float64)
    j = np.arange(M, dtype=np.float64)
    C4 = np.cos(np.pi / M * np.outer(j + 0.5, k + 0.5))          # (M j, M k)
    C4s = C4.copy()
    C4s[:512] *= -1.0         # sign of the B-half fold
    Cm = np.cos(np.pi / M * np.outer(n + 0.5 + M / 2, k + 0.5))  # (N2, M)
    Cw2 = (2.0 / M) * (w[:, None] * Cm)                          # (N2, M)
    R1 = Cw2[:M].T        # (M k, M r): A-half  td[:, r]
    R2 = Cw2[M:].T        # (M k, M r): B-half  td[:, 1024+r]

    def lay(a):  # (1024, 1024) -> (128, 8, 1024) [p, t, c] = a[t*128+p, c]
        return np.ascontiguousarray(
            a.astype(np.float32).reshape(8, 128, M).transpose(1, 0, 2))

    g = np.arange(121, dtype=np.float64)
    consts = {
        "c4": _round_mant(lay(C4s)),
        "r1": lay(R1).astype(ml_dtypes.bfloat16),
        "r2": lay(R2).astype(ml_dtypes.bfloat16),
        "ident": np.eye(128, dtype=ml_dtypes.bfloat16),
        "iota121": np.broadcast_to(g.astype(np.float32), (128, 121)).copy(),
        "lutinv": np.broadcast_to(
            np.exp2(-0.75 * g / 4.0).astype(np.float32), (128, 121)).copy(),
    }
    return consts


def host_inputs(audio_c):
    """Per-core input prep: four windowed chunk-major arrays (2,4,128,F).

    W1F[c,q,p,f] = x[c,f,  q*128+p       ] * wa[q*128+p]
    W1R[c,q,p,f] = x[c,f,  1023-(q*128+p)] * wa[1023-(q*128+p)]
    W2F[c,q,p,f] = x[c,f+1, 512+q*128+p  ] * wb[512+q*128+p]
    W2R[c,q,p,f] = x[c,f+1, 511-(q*128+p)] * wb[511-(q*128+p)]

    s chunks (for the folded DCT-IV):
      s[:, 0:4, :]  (B-half, sign in C4s) = W2R + W2F
      s[:, 4:8, :]  (A-half)              = W1F - W1R
    """
    C, T = audio_c.shape
    F = -(-(T + M) // M)
    nrows = F + 1
    x = np.zeros((C, nrows * M), np.float32)
    x[:, M:M + T] = audio_c
    x = x.reshape(C, nrows, M)

    n = np.arange(N2, dtype=np.float64)
    w = np.sin(np.pi / N2 * (n + 0.5)).astype(np.float32)
    wa, wb = w[:M], w[M:]

    xc = x[:, :F, :]          # (C, F, 1024) frame f <- row f
    xn = x[:, 1:F + 1, :]     # frame f <- row f+1

    def chunkmajor(a):        # (C, F, 512) -> (C, 4, 128, F)
        return np.ascontiguousarray(
            a.reshape(C, F, 4, 128).transpose(0, 2, 3, 1))

    w1f = chunkmajor(xc[:, :, :512] * wa[:512])
    w1r = chunkmajor(xc[:, :, ::-1][:, :, :512] * wa[::-1][:512])
    w2f = chunkmajor(xn[:, :, 512:] * wb[512:])
    w2r = chunkmajor((xn[:, :, ::-1] * wb[::-1])[:, :, 512:])
    return {"w1f": w1f, "w1r": w1r, "w2f": w2f, "w2r": w2r}, F


def build_nc(nb, ncores=NCORES):
    """Build the per-core Bass kernel. nb: number of 128-frame blocks."""
    F = nb * 128
    out_len = F * M

    nc = bacc.Bacc("TRN2", target_bir_lowering=False, debug=False,
                   num_devices=ncores)
    f32 = mybir.dt.float32
    f32r = mybir.dt.float32r
    bf16 = mybir.dt.bfloat16
    i32 = mybir.dt.int32
    i16 = mybir.dt.int16
    i8 = mybir.dt.int8
    Alu = mybir.AluOpType
    Act = mybir.ActivationFunctionType

    w1f_d = nc.dram_tensor("w1f", [2, 4, 128, F], f32, kind="ExternalInput")
    w1r_d = nc.dram_tensor("w1r", [2, 4, 128, F], f32, kind="ExternalInput")
    w2f_d = nc.dram_tensor("w2f", [2, 4, 128, F], f32, kind="ExternalInput")
    w2r_d = nc.dram_tensor("w2r", [2, 4, 128, F], f32, kind="ExternalInput")
    c4_d = nc.dram_tensor("c4", [128, 8, M], f32r, kind="ExternalInput")
    r1_d = nc.dram_tensor("r1", [128, 8, M], bf16, kind="ExternalInput")
    r2_d = nc.dram_tensor("r2", [128, 8, M], bf16, kind="ExternalInput")
    id_d = nc.dram_tensor("ident", [128, 128], bf16, kind="ExternalInput")
    io_d = nc.dram_tensor("iota121", [128, 121], f32, kind="ExternalInput")
    lv_d = nc.dram_tensor("lutinv", [128, 121], f32, kind="ExternalInput")
    out_d = nc.dram_tensor("out", [2, out_len], f32, kind="ExternalOutput")

    def w_slice(t, c, b):
        # (128, 4, 128) tile: [p, q, f] = t[c, q, p, b*128+f]
        return bass.AP(tensor=t, offset=(c * 4 * 128 + 0) * F + b * 128,
                       ap=[[F, 128], [128 * F, 4], [1, 128]])

    def out_slice(c, blk0, npart, r0, nr):
        return bass.AP(tensor=out_d, offset=c * out_len + blk0 * M + r0,
                       ap=[[M, npart], [1, nr]])

    # integer threshold: bits > TARGET  <=>  sum(E) > TARGET + 125*2048
    thresh = float(int(np.floor(TARGET_BITS + 125 * 2048))) + 0.5  # 258730.5

    with tile.TileContext(nc) as tc:
        import contextlib
        ctx = contextlib.ExitStack()
        with ctx:
            consts = ctx.enter_context(tc.tile_pool(name="consts", bufs=1))
            xin = ctx.enter_context(tc.tile_pool(name="xin", bufs=2))
            srp = ctx.enter_context(tc.tile_pool(name="srp", bufs=2))
            abp = ctx.enter_context(tc.tile_pool(name="abp", bufs=2))
            axp = ctx.enter_context(tc.tile_pool(name="axp", bufs=3))
            sbp = ctx.enter_context(tc.tile_pool(name="sbp", bufs=2))
            zp = ctx.enter_context(tc.tile_pool(name="zp", bufs=2))
            escr = ctx.enter_context(tc.tile_pool(name="escr", bufs=2))
            scr = ctx.enter_context(tc.tile_pool(name="scr", bufs=3))
            ax0p = ctx.enter_context(tc.tile_pool(name="ax0p", bufs=2))
            dfp = ctx.enter_context(tc.tile_pool(name="dfp", bufs=2))
            dqtp = ctx.enter_context(tc.tile_pool(name="dqtp", bufs=2))
            outp = ctx.enter_context(tc.tile_pool(name="outp", bufs=2))
            stat = ctx.enter_context(tc.tile_pool(name="stat", bufs=5))
            lutp = ctx.enter_context(tc.tile_pool(name="lutp", bufs=2))
            psM = ctx.enter_context(tc.tile_pool(name="psM", bufs=3, space="PSUM"))
            psQ = ctx.enter_context(tc.tile_pool(name="psQ", bufs=2, space="PSUM"))
            psI = ctx.enter_context(tc.tile_pool(name="psI", bufs=2, space="PSUM"))

            c4_sb = consts.tile([128, 8, M], f32r)
            nc.sync.dma_start(out=c4_sb[:, 0:4, :], in_=c4_d[:, 0:4, :])
            nc.sync.dma_start(out=c4_sb[:, 4:8, :], in_=c4_d[:, 4:8, :])
            r1_sb = consts.tile([128, 8, M], bf16)
            r2_sb = consts.tile([128, 8, M], bf16)
            id_sb = consts.tile([128, 128], bf16)
            nc.sync.dma_start(out=id_sb, in_=id_d[:, :])
            io_sb = consts.tile([128, 121], f32)
            nc.sync.dma_start(out=io_sb, in_=io_d[:, :])
            lv_sb = consts.tile([128, 121], f32)
            nc.sync.dma_start(out=lv_sb, in_=lv_d[:, :])
            eps35 = consts.tile([128, 1], f32)
            nc.vector.memset(eps35, 1e-35)
            zbf = consts.tile([128, 1], bf16)
            nc.vector.memset(zbf, 0.0)
            half05 = consts.tile([128, 1], f32)
            nc.vector.memset(half05, 0.5)
            zf32 = consts.tile([128, 1], f32)
            nc.vector.memset(zf32, 0.0)
            magicb = consts.tile([128, 1], f32)
            nc.vector.memset(magicb, MAGIC)

            # dqT ring: [parity][channel] -> tile (128, 8, 129) bf16
            dqt_ring = [[None, None], [None, None]]

            def mdct_block(b):
                """Returns (sb2, ax2) fused tiles (128, 2, M) for block b."""
                sb2 = sbp.tile([128, 2, M], bf16, name=f"sb_{b}", tag="sb")
                ab2 = abp.tile([128, 2, M], f32, name=f"ab_{b}", tag="ab")
                for c in range(2):
                    tw1f = xin.tile([128, 4, 128], f32, name=f"w1f_{b}_{c}",
                                    tag="xin")
                    nc.sync.dma_start(out=tw1f, in_=w_slice(w1f_d, c, b))
                    tw1r = xin.tile([128, 4, 128], f32, name=f"w1r_{b}_{c}",
                                    tag="xin")
                    nc.sync.dma_start(out=tw1r, in_=w_slice(w1r_d, c, b))
                    tw2f = xin.tile([128, 4, 128], f32, name=f"w2f_{b}_{c}",
                                    tag="xin")
                    nc.sync.dma_start(out=tw2f, in_=w_slice(w2f_d, c, b))
                    tw2r = xin.tile([128, 4, 128], f32, name=f"w2r_{b}_{c}",
                                    tag="xin")
                    nc.sync.dma_start(out=tw2r, in_=w_slice(w2r_d, c, b))

                    sr = srp.tile([128, 8, 128], f32r, name=f"sr_{b}_{c}",
                                  tag="sr")
                    nc.vector.tensor_add(out=sr[:, 0:4, :], in0=tw2r, in1=tw2f)
                    nc.vector.tensor_sub(out=sr[:, 4:8, :], in0=tw1f, in1=tw1r)

                    for kc in range(2):
                        psm = psM.tile([128, 512], f32, name=f"psm_{b}_{c}_{kc}",
                                       tag="psm")
                        for jt in range(8):
                            nc.tensor.matmul(psm, sr[:, jt, :],
                                             c4_sb[:, jt, kc * 512:(kc + 1) * 512],
                                             start=(jt == 0), stop=(jt == 7))
                        ks = slice(kc * 512, (kc + 1) * 512)
                        nc.scalar.activation(out=ab2[:, c, ks], in_=psm,
                                             func=Act.Abs)
                        nc.scalar.activation(out=sb2[:, c, ks], in_=psm,
                                             func=Act.Sign, bias=zf32)

                ln = scr.tile([128, 2, M], f32, name=f"ln_{b}", tag="scr")
                nc.scalar.activation(out=ln, in_=ab2, func=Act.Ln, bias=eps35)
                ax2 = axp.tile([128, 2, M], f32, name=f"ax_{b}", tag="ax")
                if NEWTON:
                    ax0 = ax0p.tile([128, 2, M], f32, name=f"ax0_{b}",
                                    tag="ax0")
                    nc.scalar.activation(out=ax0, in_=ln, func=Act.Exp,
                                         scale=0.75)
                    # one Newton step on a^4 = |c|^3:
                    #   a' = 0.75 a + 0.25 (|c|/a)^3
                    rcp = scr.tile([128, 2, M], f32, name=f"rcp_{b}",
                                   tag="scr")
                    nc.vector.reciprocal(out=rcp, in_=ax0)
                    tt = scr.tile([128, 2, M], f32, name=f"tt_{b}", tag="scr")
                    nc.gpsimd.tensor_mul(out=tt, in0=ab2, in1=rcp)
                    t2 = scr.tile([128, 2, M], f32, name=f"t2_{b}", tag="scr")
                    nc.scalar.activation(out=t2, in_=tt, func=Act.Square)
                    v3 = scr.tile([128, 2, M], f32, name=f"v3_{b}", tag="scr")
                    nc.vector.scalar_tensor_tensor(out=v3, in0=t2, scalar=0.25,
                                                   in1=tt, op0=Alu.mult,
                                                   op1=Alu.mult)
                    nc.vector.scalar_tensor_tensor(out=ax2, in0=ax0,
                                                   scalar=0.75, in1=v3,
                                                   op0=Alu.mult, op1=Alu.add)
                else:
                    ax0 = ax2
                    nc.scalar.activation(out=ax2, in_=ln, func=Act.Exp,
                                         scale=0.75)
                return sb2, ax2, ax0

            def lut_pow2(prefix, b, g_ap, tag):
                """inv = 2^{-3 g/16} exactly via one-hot LUT: 2 DVE ops."""
                eq = lutp.tile([128, 121], f32, name=f"{prefix}eq_{b}",
                               tag=f"{tag}a")
                nc.vector.tensor_scalar(out=eq, in0=io_sb, scalar1=g_ap,
                                        scalar2=None, op0=Alu.is_equal)
                eqs = lutp.tile([128, 121], f32, name=f"{prefix}es_{b}",
                                tag=f"{tag}b")
                nc.vector.tensor_mul(out=eqs, in0=eq, in1=lv_sb)
                inv = stat.tile([128, 1], f32, name=f"{prefix}iv_{b}",
                                tag=f"{tag}c")
                with nc.allow_low_precision(reason="one-hot dot"):
                    nc.vector.tensor_scalar(
                        out=eqs, in0=eqs, scalar1=1.0, scalar2=None,
                        op0=Alu.mult, op1=Alu.add, accum_out=inv)
                return inv

            def search_block(b, ax2, ax0):
                """8-iter integer binary search; returns gains (hi) (128,1)."""
                lo = stat.tile([128, 1], f32, name=f"lo_{b}", tag="lo")
                nc.vector.memset(lo, 0.0)
                hi = stat.tile([128, 1], f32, name=f"hi_{b}", tag="hi")
                nc.vector.memset(hi, 120.0)
                for it in range(8):
                    axu = ax0 if it < 2 else ax2
                    t = stat.tile([128, 1], f32, name=f"t_{b}_{it}", tag="st1")
                    nc.vector.tensor_add(out=t, in0=lo, in1=hi)
                    mid = stat.tile([128, 1], f32, name=f"mid_{b}_{it}",
                                    tag="st2")
                    nc.vector.tensor_scalar(out=mid, in0=t, scalar1=0.5,
                                            scalar2=-0.25, op0=Alu.mult,
                                            op1=Alu.add)
                    nc.vector.tensor_scalar(out=mid, in0=mid, scalar1=MAGIC,
                                            scalar2=MAGIC, op0=Alu.add,
                                            op1=Alu.subtract)
                    inv = lut_pow2(f"s{it}_", b, mid, "lp")
                    z = zp.tile([128, 2, M], f32, name=f"z_{b}_{it}", tag="z")
                    ezt = escr.tile([128, 2, M], i16, name=f"e_{b}_{it}",
                                    tag="ez")
                    # z halves off-DVE (Pool / ACT); exponent extraction =
                    # high-i16 halves >> 7, then one fused arith accumulate,
                    # both on DVE (Pool has no shift; bitwise can't fuse
                    # with the arith reduce in one op).
                    nc.gpsimd.tensor_scalar(out=z[:, 0, :], in0=axu[:, 0, :],
                                            scalar1=inv, scalar2=0.5,
                                            op0=Alu.mult, op1=Alu.add)
                    nc.scalar.activation(out=z[:, 1, :], in_=axu[:, 1, :],
                                         func=Act.Identity, scale=inv,
                                         bias=half05)
                    zi16 = z.bitcast(i16)   # (128, 2, 2M)
                    es = stat.tile([128, 1], f32, name=f"es_{b}_{it}",
                                   tag="es")
                    with nc.allow_low_precision(reason="exponent bits"):
                        nc.vector.tensor_scalar(
                            out=ezt, in0=zi16[:, :, 1:2 * M:2],
                            scalar1=7, scalar2=None,
                            op0=Alu.logical_shift_right)
                        nc.vector.tensor_scalar(
                            out=ezt, in0=ezt, scalar1=1,
                            scalar2=None, op0=Alu.mult, op1=Alu.add,
                            accum_out=es)
                    msk = stat.tile([128, 1], i32, name=f"mk_{b}_{it}",
                                    tag="st6")
                    with nc.allow_low_precision(reason="int mask"):
                        nc.vector.tensor_scalar(out=msk, in0=es,
                                                scalar1=thresh,
                                                scalar2=None, op0=Alu.is_gt)
                        mskn = stat.tile([128, 1], i32, name=f"mn_{b}_{it}",
                                         tag="st7")
                        nc.vector.tensor_scalar(out=mskn, in0=msk, scalar1=-1,
                                                scalar2=1, op0=Alu.mult,
                                                op1=Alu.add)
                    mp1 = stat.tile([128, 1], f32, name=f"mp_{b}_{it}",
                                    tag="st8")
                    nc.vector.tensor_scalar(out=mp1, in0=mid, scalar1=1.0,
                                            scalar2=None, op0=Alu.add)
                    # lo = too_big ? mid+1 : lo ; hi = too_big ? hi : mid
                    nc.vector.copy_predicated(out=lo, mask=msk, data=mp1)
                    nc.vector.copy_predicated(out=hi, mask=mskn, data=mid)
                return hi

            def quant_block(b, gains, ax2, sb2):
                """Quantize+dequantize; returns fused df (128, 2, M) i32
                (= fp32 bit pattern of signed dq).

                q_soft = ax75 * 2^{-3g/16}; q1 = q_soft + MAGIC holds
                MAGIC + round(q_soft) exactly. a43 = qm^{4/3} * 2^{g/4} via
                one Exp with per-partition bias g*ln2/4."""
                inv2 = lut_pow2("q_", b, gains, "lq")
                gb = stat.tile([128, 1], f32, name=f"gb_{b}", tag="st5")
                nc.vector.tensor_scalar(out=gb, in0=gains, scalar1=LN2 / 4.0,
                                        scalar2=None, op0=Alu.mult)
                q1 = scr.tile([128, 2, M], f32, name=f"q1_{b}", tag="scr")
                nc.scalar.activation(out=q1, in_=ax2, func=Act.Identity,
                                     scale=inv2, bias=magicb)
                qm = scr.tile([128, 2, M], f32, name=f"qm_{b}", tag="scr")
                nc.vector.tensor_scalar(out=qm, in0=q1, scalar1=MAGIC,
                                        scalar2=0.5, op0=Alu.subtract,
                                        op1=Alu.max)
                lq = scr.tile([128, 2, M], f32, name=f"lq_{b}", tag="scr")
                nc.scalar.activation(out=lq, in_=qm, func=Act.Ln)
                a43 = scr.tile([128, 2, M], f32, name=f"a43_{b}", tag="scr")
                nc.scalar.activation(out=a43, in_=lq, func=Act.Exp,
                                     scale=4.0 / 3.0, bias=gb)
                mq = scr.tile([128, 2, M], f32, name=f"mq_{b}", tag="scr")
                nc.gpsimd.tensor_scalar(out=mq, in0=q1, scalar1=MAGIC + 0.5,
                                        scalar2=None, op0=Alu.is_gt)
                d2 = scr.tile([128, 2, M], f32, name=f"d2_{b}", tag="scr")
                nc.gpsimd.tensor_mul(out=d2, in0=a43, in1=mq)
                df = dfp.tile([128, 2, M], bf16, name=f"df_{b}", tag="df")
                nc.gpsimd.tensor_mul(out=df, in0=d2, in1=sb2)
                return df

            def dqt_block(b, df):
                """Transpose signed dq into the bf16 dqT ring; write sliver
                col 128 of block b-1's buffers."""
                par = b % 2
                dqf = df
                for c in range(2):
                    buf = dqtp.tile([128, 8, 129], bf16, name=f"dqt_{b}_{c}",
                                    tag=f"dqt{c}")
                    dqt_ring[par][c] = buf
                    for kt in range(8):
                        psq = psQ.tile([128, 128], bf16, name=f"psq_{b}_{c}_{kt}",
                                       tag="psq")
                        nc.tensor.transpose(
                            psq, dqf[:, c, kt * 128:(kt + 1) * 128], id_sb)
                        if kt % 2 == 0:
                            nc.vector.tensor_copy(out=buf[:, kt, 0:128],
                                                  in_=psq)
                        else:
                            nc.scalar.activation(out=buf[:, kt, 0:128],
                                                 in_=psq, func=Act.Copy)
                        if b > 0:
                            prev = dqt_ring[1 - par][c]
                            nc.scalar.activation(out=prev[:, kt, 128:129],
                                                 in_=psq[:, 0:1],
                                                 func=Act.Copy)

            def imdct_block(bp):
                """IMDCT + fused OLA for out blocks [bp*128, bp*128+128)."""
                par = bp % 2
                for c in range(2):
                    buf = dqt_ring[par][c]
                    for rc in range(2):
                        psr = psI.tile([128, 512], f32, name=f"psr_{bp}_{c}_{rc}",
                                       tag="psr")
                        for kt in range(8):
                            nc.tensor.matmul(
                                psr, buf[:, kt, 0:128],
                                r2_sb[:, kt, rc * 512:(rc + 1) * 512],
                                start=(kt == 0), stop=False)
                        for kt in range(8):
                            nc.tensor.matmul(
                                psr, buf[:, kt, 1:129],
                                r1_sb[:, kt, rc * 512:(rc + 1) * 512],
                                start=False, stop=(kt == 7))
                        ot = outp.tile([128, 512], f32, name=f"ot_{bp}_{c}_{rc}",
                                       tag="ot")
                        nc.scalar.activation(out=ot, in_=psr, func=Act.Copy)
                        nc.sync.dma_start(
                            out=out_slice(c, bp * 128, 128, rc * 512, 512),
                            in_=ot)

            # software pipeline: emit block b+1's front-end before block
            # b's back-end so the in-order PE queue isn't head-of-line
            # blocked by imdct(b-1) (which waits on block b's sliver).
            state = mdct_block(0)
            # IMDCT matrices aren't needed until imdct(0) at iteration 1;
            # keep their 12us of DMA behind the first block's inputs.
            nc.sync.dma_start(out=r1_sb, in_=r1_d[:, :, :])
            nc.sync.dma_start(out=r2_sb, in_=r2_d[:, :, :])
            for b in range(nb):
                nxt = mdct_block(b + 1) if b + 1 < nb else None
                sb2, ax2, ax0 = state
                gains = search_block(b, ax2, ax0)
                df = quant_block(b, gains, ax2, sb2)
                dqt_block(b, df)
                if b > 0:
                    imdct_block(b - 1)
                state = nxt
            # final sliver = 0 (frame F does not exist), then last IMDCT
            par = (nb - 1) % 2
            for c in range(2):
                for kt in range(8):
                    nc.gpsimd.tensor_copy(out=dqt_ring[par][c][:, kt, 128:129],
                                          in_=zbf)
            imdct_block(nb - 1)

    nc.compile()
    return nc


_CACHE = {}


def _get_nc(nb, ncores):
    key = (nb, ncores)
    if key not in _CACHE:
        _CACHE[key] = (build_nc(nb, ncores), host_constants())
    return _CACHE[key]


def run(audio, trace=False):
    """audio (B, C, T) float32 -> (out (B, C, T) float32, results obj)."""
    B, C, T = audio.shape
    assert C == 2
    F = -(-(T + M) // M)
    nb = F // 128
    assert nb * 128 == F, "frame count must be a multiple of 128"

    nc, consts = _get_nc(nb, B)

    audio = np.ascontiguousarray(audio, np.float32)
    in_maps = []
    for core in range(B):
        arrs, _ = host_inputs(audio[core])
        in_maps.append({**arrs, **consts})

    res = run_bass_kernel_spmd(nc, in_maps, core_ids=list(range(B)),
                               trace=trace)
    out = np.stack([r["out"][:, :T] for r in res.results])
    return out, res


def kernel(audio):
    return run(audio)[0]
